# revision 9
# baseline (speedup 1.0000x reference)
"""Trainium2 Bass kernel for nn_MoERegressor (MoE routing, dense combine).

Contract: kernel(**inputs) takes the FULL unsharded inputs (keyed as in
setup_inputs()) and returns the full (out, gating_weights) pair, matching
reference(). Internally: pure data parallel over 8 NeuronCores (batch dim
sharded), all params replicated; a Bass/Tile kernel does the whole fused
computation on each core.

Self-contained: hardcodes shapes; only imports the Bass stack from
/opt/trn_rl_repo.
"""

import sys

for _p in ("/opt/trn_rl_repo",):
    if _p not in sys.path:
        sys.path.insert(0, _p)

import numpy as np

# ---- problem constants (hardcoded from the problem spec) ----
B = 262144
N_CORES = 8
D = 153            # D_IN
DK0, DK1 = 128, 25  # contraction split of D
E = 8
H1 = 128
H2 = 64
GH = 64
GH2 = 32
DO = 8             # D_OUT
EPS = 1e-5

P = 128            # SBUF partitions
NB = 512           # batch columns per chunk (fp32 max moving free dim)
J = NB // P        # row-tiles per chunk
ROWS = B // N_CORES
N_CHUNKS = ROWS // NB


def _build_program(rows, repeat, flags):
    """Emit the Bass/Tile program for one core processing `rows` rows.

    flags: (in_affine, bg3_any, be3_any, out_affine) -- enable the general
    paths for the (normally identity/zero) LN affines and biases.
    """
    import concourse.bacc as bacc
    import concourse.tile as tile
    from concourse import mybir

    in_affine, bg3_any, be3_any, out_affine = flags
    n_chunks = rows // NB
    assert rows % NB == 0

    f32 = mybir.dt.float32
    AF = mybir.ActivationFunctionType
    ALU = mybir.AluOpType
    AX = mybir.AxisListType

    nc = bacc.Bacc("TRN2", target_bir_lowering=False)

    # ---- DRAM I/O ----
    x_d = nc.dram_tensor("x", [rows, D], f32, kind="ExternalInput")
    wp_d = nc.dram_tensor("wp", [D, D], f32, kind="ExternalInput")
    wg1_d = nc.dram_tensor("wg1", [D, GH], f32, kind="ExternalInput")
    wg2_d = nc.dram_tensor("wg2", [GH, GH2], f32, kind="ExternalInput")
    wg3_d = nc.dram_tensor("wg3", [GH2, E], f32, kind="ExternalInput")
    we1_d = nc.dram_tensor("we1", [E, D, H1], f32, kind="ExternalInput")
    we2_d = nc.dram_tensor("we2", [E, H1, H2], f32, kind="ExternalInput")
    we3bd_d = nc.dram_tensor("we3bd", [E // 2, H1, 32], f32, kind="ExternalInput")
    bp_d = nc.dram_tensor("bp", [D, 1], f32, kind="ExternalInput")
    bg1_d = nc.dram_tensor("bg1", [GH, 1], f32, kind="ExternalInput")
    bg2_d = nc.dram_tensor("bg2", [GH2, 1], f32, kind="ExternalInput")
    be1_d = nc.dram_tensor("be1", [E, H1], f32, kind="ExternalInput")
    be2s_d = nc.dram_tensor("be2s", [E // 2, P], f32, kind="ExternalInput")
    r_d = nc.dram_tensor("rmat", [E, P], f32, kind="ExternalInput")
    s_d = nc.dram_tensor("smat", [P, DO], f32, kind="ExternalInput")
    id_d = nc.dram_tensor("ident", [P, P], f32, kind="ExternalInput")
    if in_affine:
        ging_d = nc.dram_tensor("ging", [1, D], f32, kind="ExternalInput")
        ginb_d = nc.dram_tensor("ginb", [1, D], f32, kind="ExternalInput")
    if bg3_any:
        bg3_d = nc.dram_tensor("bg3", [1, E], f32, kind="ExternalInput")
    if be3_any:
        be3_d = nc.dram_tensor("be3", [E, DO], f32, kind="ExternalInput")
    if out_affine:
        gog_d = nc.dram_tensor("gog", [1, DO], f32, kind="ExternalInput")
        gob_d = nc.dram_tensor("gob", [1, DO], f32, kind="ExternalInput")

    out_d = nc.dram_tensor("out", [rows, DO], f32, kind="ExternalOutput")
    gw_d = nc.dram_tensor("gw", [rows, DO], f32, kind="ExternalOutput")

    from contextlib import ExitStack

    with tile.TileContext(nc) as tc, ExitStack() as ctx:
        # ---- pools ----
        consts = ctx.enter_context(tc.tile_pool(name="consts", bufs=1))
        # PSUM pools (8 banks total):
        pp128 = ctx.enter_context(tc.tile_pool(name="pp128", bufs=2, space="PSUM"))
        pp25 = ctx.enter_context(tc.tile_pool(name="pp25", bufs=1, space="PSUM"))
        ppg = ctx.enter_context(tc.tile_pool(name="ppg", bufs=2, space="PSUM"))
        ppeo = ctx.enter_context(tc.tile_pool(name="ppeo", bufs=1, space="PSUM"))
        ppsm = ctx.enter_context(tc.tile_pool(name="ppsm", bufs=1, space="PSUM"))
        # SBUF working pools
        sbx = ctx.enter_context(tc.tile_pool(name="sbx", bufs=2))
        sbxn = ctx.enter_context(tc.tile_pool(name="sbxn", bufs=2))
        sbxt = ctx.enter_context(tc.tile_pool(name="sbxt", bufs=2))
        sbproj = ctx.enter_context(tc.tile_pool(name="sbproj", bufs=2))
        sbg = ctx.enter_context(tc.tile_pool(name="sbg", bufs=2))
        sbh1 = ctx.enter_context(tc.tile_pool(name="sbh1", bufs=3))
        sbh2 = ctx.enter_context(tc.tile_pool(name="sbh2", bufs=2))
        sbsm = ctx.enter_context(tc.tile_pool(name="sbsm", bufs=3))
        sbst = ctx.enter_context(tc.tile_pool(name="sbst", bufs=4))

        # ---- load constants / weights ----
        wp0 = consts.tile([DK0, D], f32)
        nc.sync.dma_start(out=wp0, in_=wp_d[0:DK0, :])
        wp1 = consts.tile([DK1, D], f32)
        nc.sync.dma_start(out=wp1, in_=wp_d[DK0:D, :])
        we1a = consts.tile([DK0, E, H1], f32)
        nc.sync.dma_start(out=we1a, in_=we1_d.rearrange("e k m -> k e m")[0:DK0])
        we1b = consts.tile([DK1, E, H1], f32)
        nc.sync.dma_start(out=we1b, in_=we1_d.rearrange("e k m -> k e m")[DK0:D])
        we2s = consts.tile([H1, E, H2], f32)
        nc.sync.dma_start(out=we2s, in_=we2_d.rearrange("e k m -> k e m"))
        we3bd = consts.tile([H1, E // 2, 32], f32)
        nc.sync.dma_start(out=we3bd, in_=we3bd_d.rearrange("p k m -> k p m"))
        wg1a = consts.tile([DK0, GH], f32)
        nc.sync.dma_start(out=wg1a, in_=wg1_d[0:DK0, :])
        wg1b = consts.tile([DK1, GH], f32)
        nc.sync.dma_start(out=wg1b, in_=wg1_d[DK0:D, :])
        wg2s = consts.tile([GH, GH2], f32)
        nc.sync.dma_start(out=wg2s, in_=wg2_d[:, :])
        wg3s = consts.tile([GH2, E], f32)
        nc.sync.dma_start(out=wg3s, in_=wg3_d[:, :])
        rs = consts.tile([E, P], f32)
        nc.sync.dma_start(out=rs, in_=r_d[:, :])
        ss0 = consts.tile([64, DO], f32)
        nc.sync.dma_start(out=ss0, in_=s_d[0:64, :])
        ss1 = consts.tile([64, DO], f32)
        nc.sync.dma_start(out=ss1, in_=s_d[64:P, :])
        ident = consts.tile([P, P], f32)
        nc.sync.dma_start(out=ident, in_=id_d[:, :])
        bpc0 = consts.tile([DK0, 1], f32)
        nc.sync.dma_start(out=bpc0, in_=bp_d[0:DK0, :])
        bpc1 = consts.tile([DK1, 1], f32)
        nc.sync.dma_start(out=bpc1, in_=bp_d[DK0:D, :])
        bg1c = consts.tile([GH, 1], f32)
        nc.sync.dma_start(out=bg1c, in_=bg1_d[:, :])
        bg2c = consts.tile([GH2, 1], f32)
        nc.sync.dma_start(out=bg2c, in_=bg2_d[:, :])
        be1c = consts.tile([H1, E], f32)
        nc.sync.dma_start(out=be1c, in_=be1_d.rearrange("e k -> k e"))
        be2c = consts.tile([P, E // 2], f32)
        nc.sync.dma_start(out=be2c, in_=be2s_d.rearrange("p k -> k p"))
        if in_affine:
            ging = consts.tile([P, D], f32)
            nc.sync.dma_start(out=ging, in_=ging_d[:, :].to_broadcast((P, D)))
            ginb = consts.tile([P, D], f32)
            nc.sync.dma_start(out=ginb, in_=ginb_d[:, :].to_broadcast((P, D)))
        if bg3_any:
            bg3b = consts.tile([P, E], f32)
            nc.sync.dma_start(out=bg3b, in_=bg3_d[:, :].to_broadcast((P, E)))
        if be3_any:
            be3s = consts.tile([E, DO], f32)
            nc.sync.dma_start(out=be3s, in_=be3_d[:, :])
        if out_affine:
            gog = consts.tile([P, DO], f32)
            nc.sync.dma_start(out=gog, in_=gog_d[:, :].to_broadcast((P, DO)))
            gob = consts.tile([P, DO], f32)
            nc.sync.dma_start(out=gob, in_=gob_d[:, :].to_broadcast((P, DO)))

        eps_c = consts.tile([P, 1], f32)
        nc.vector.memset(eps_c, EPS)

        BNS = nc.vector.BN_STATS_DIM
        BNA = nc.vector.BN_AGGR_DIM

        def chunk_body(c):
            b0 = c * NB
            xr_view = x_d[b0:b0 + NB, :].rearrange("(j p) d -> p j d", p=P)
            # -------- load + input layernorm (row-major) --------
            x_row = sbx.tile([P, J, D], f32, tag="xrow")
            nc.sync.dma_start(out=x_row, in_=xr_view)

            stats = sbst.tile([P, J, BNS], f32, tag="stats")
            mv = sbst.tile([P, J, BNA], f32, tag="mv")
            for j in range(J):
                nc.vector.bn_stats(out=stats[:, j, :], in_=x_row[:, j, :])
                nc.vector.bn_aggr(out=mv[:, j, :], in_=stats[:, j, :])
            sd4 = sbst.tile([P, J], f32, tag="sd4")
            # sd = sqrt(var + eps)
            nc.scalar.activation(out=sd4, in_=mv[:, :, 1], func=AF.Sqrt,
                                 bias=eps_c, scale=1.0)
            rstd4 = sbst.tile([P, J], f32, tag="rstd4")
            nc.vector.reciprocal(out=rstd4, in_=sd4)
            xn_row = sbxn.tile([P, J, D], f32, tag="xnrow")
            for j in range(J):
                nc.vector.tensor_scalar(
                    out=xn_row[:, j, :], in0=x_row[:, j, :],
                    scalar1=mv[:, j, 0:1], scalar2=rstd4[:, j:j + 1],
                    op0=ALU.subtract, op1=ALU.mult)
            if in_affine:
                nc.vector.tensor_mul(out=xn_row, in0=xn_row,
                                     in1=ging[:, None, :].broadcast_to((P, J, D)))
                nc.vector.tensor_add(out=xn_row, in0=xn_row,
                                     in1=ginb[:, None, :].broadcast_to((P, J, D)))

            # -------- transpose x_norm to feature-major --------
            ptx = pp128.tile([P, NB], f32, tag="p128")
            for j in range(J):
                nc.tensor.transpose(ptx[:, j * P:(j + 1) * P],
                                    xn_row[:, j, 0:DK0], ident)
            ptx25 = pp25.tile([DK1, NB], f32, tag="p25")
            for j in range(J):
                nc.tensor.transpose(ptx25[:, j * P:(j + 1) * P],
                                    xn_row[:, j, DK0:D], ident)
            xt = sbxt.tile([P, NB], f32, tag="xt")
            nc.vector.tensor_copy(out=xt, in_=ptx)
            xt25 = sbxt.tile([DK1, NB], f32, tag="xt25")
            nc.vector.tensor_copy(out=xt25, in_=ptx25)

            # -------- x_proj = relu(xn @ Wp + bp) + xn  (feature-major) ----
            pwp = pp128.tile([P, NB], f32, tag="p128")
            nc.tensor.matmul(pwp, wp0[:, 0:DK0], xt, start=True, stop=False)
            nc.tensor.matmul(pwp, wp1[:, 0:DK0], xt25, start=False, stop=True)
            pwp25 = pp25.tile([DK1, NB], f32, tag="p25")
            nc.tensor.matmul(pwp25, wp0[:, DK0:D], xt, start=True, stop=False)
            nc.tensor.matmul(pwp25, wp1[:, DK0:D], xt25, start=False, stop=True)
            xproj = sbproj.tile([P, NB], f32, tag="xproj")
            nc.scalar.activation(out=xproj, in_=pwp, func=AF.Relu, bias=bpc0)
            nc.vector.tensor_add(out=xproj, in0=xproj, in1=xt)
            xproj25 = sbproj.tile([DK1, NB], f32, tag="xproj25")
            nc.scalar.activation(out=xproj25, in_=pwp25, func=AF.Relu, bias=bpc1)
            nc.vector.tensor_add(out=xproj25, in0=xproj25, in1=xt25)

            # -------- gating MLP --------
            pg1 = ppg.tile([GH, NB], f32, tag="pg")
            nc.tensor.matmul(pg1, wg1a, xproj, start=True, stop=False)
            nc.tensor.matmul(pg1, wg1b, xproj25, start=False, stop=True)
            g1 = sbg.tile([GH, NB], f32, tag="g1")
            nc.scalar.activation(out=g1, in_=pg1, func=AF.Relu, bias=bg1c)
            pg2 = ppg.tile([GH2, NB], f32, tag="pg")
            nc.tensor.matmul(pg2, wg2s, g1, start=True, stop=True)
            g2 = sbg.tile([GH2, NB], f32, tag="g2")
            nc.scalar.activation(out=g2, in_=pg2, func=AF.Relu, bias=bg2c)
            # logits, row-major [128, j, 8] via flipped matmuls
            plog = ppsm.tile([P, J, DO], f32, tag="psm")
            for j in range(J):
                nc.tensor.matmul(plog[:, j, :], g2[:, j * P:(j + 1) * P], wg3s,
                                 start=True, stop=True)
            # softmax over E=8 (grouped, row-major)
            if bg3_any:
                logit = sbsm.tile([P, J, DO], f32, tag="sm1")
                nc.vector.tensor_add(out=logit, in0=plog,
                                     in1=bg3b[:, None, :].broadcast_to((P, J, DO)))
            else:
                logit = plog
            mx4 = sbst.tile([P, J], f32, tag="mx4")
            nc.vector.reduce_max(out=mx4, in_=logit, axis=AX.X)
            sh = sbsm.tile([P, J, DO], f32, tag="sm2")
            nc.vector.tensor_sub(out=sh, in0=logit,
                                 in1=mx4[:, :, None].broadcast_to((P, J, DO)))
            ex = sbsm.tile([P, J, DO], f32, tag="sm3")
            nc.scalar.activation(out=ex, in_=sh, func=AF.Exp)
            se4 = sbst.tile([P, J], f32, tag="se4")
            nc.vector.reduce_sum(out=se4, in_=ex, axis=AX.X)
            rc4 = sbst.tile([P, J], f32, tag="rc4")
            nc.vector.reciprocal(out=rc4, in_=se4)
            gw_row = sbsm.tile([P, J, DO], f32, tag="gwrow")
            nc.vector.tensor_mul(out=gw_row, in0=ex,
                                 in1=rc4[:, :, None].broadcast_to((P, J, DO)))
            nc.sync.dma_start(
                out=gw_d[b0:b0 + NB, :].rearrange("(j p) o -> p j o", p=P),
                in_=gw_row)
            # gw transposed to feature-major [8, NB]
            pgwt = ppg.tile([E, NB], f32, tag="pg")
            for j in range(J):
                nc.tensor.transpose(pgwt[:, j * P:(j + 1) * P],
                                    gw_row[:, j, :], ident)
            gwt = sbsm.tile([E, NB], f32, tag="gwt")
            nc.vector.tensor_copy(out=gwt, in_=pgwt)

            # -------- experts --------
            peo01 = ppeo.tile([2 * 32, NB], f32, tag="peo01")
            peo23 = ppeo.tile([2 * 32, NB], f32, tag="peo23")
            for pr in range(E // 2):
                h1s = []
                for which in range(2):
                    e = 2 * pr + which
                    ph1 = pp128.tile([P, NB], f32, tag="p128")
                    nc.tensor.matmul(ph1, we1a[:, e, :], xproj,
                                     start=True, stop=False)
                    nc.tensor.matmul(ph1, we1b[:, e, :], xproj25,
                                     start=False, stop=True)
                    h1 = sbh1.tile([P, NB], f32, tag="h1")
                    if e % 3 == 0:
                        nc.scalar.activation(out=h1, in_=ph1, func=AF.Relu,
                                             bias=be1c[:, e:e + 1])
                    else:
                        nc.vector.tensor_scalar(
                            out=h1, in0=ph1, scalar1=be1c[:, e:e + 1],
                            scalar2=0.0, op0=ALU.add, op1=ALU.max)
                    h1s.append(h1)
                ph2 = pp128.tile([P, NB], f32, tag="p128")
                for which in range(2):
                    e = 2 * pr + which
                    nc.tensor.matmul(ph2[which * H2:(which + 1) * H2, :],
                                     we2s[:, e, :], h1s[which],
                                     start=True, stop=True)
                h2 = sbh2.tile([P, NB], f32, tag="h2")
                if pr % 2 == 0:
                    nc.scalar.activation(out=h2, in_=ph2, func=AF.Relu,
                                         bias=be2c[:, pr:pr + 1])
                else:
                    nc.vector.tensor_scalar(
                        out=h2, in0=ph2, scalar1=be2c[:, pr:pr + 1],
                        scalar2=0.0, op0=ALU.add, op1=ALU.max)
                peo_half = peo01 if pr < 2 else peo23
                off = (pr % 2) * 32
                nc.tensor.matmul(peo_half[off:off + 32, :],
                                 we3bd[:, pr, :], h2, start=True, stop=True)

            # -------- combine: out_row = sum_e gw_e * eo_e --------
            pgrep01 = ppg.tile([2 * 32, NB], f32, tag="pg")
            nc.tensor.matmul(pgrep01, rs[:, 0:64], gwt, start=True, stop=True)
            pgrep23 = ppg.tile([2 * 32, NB], f32, tag="pg")
            nc.tensor.matmul(pgrep23, rs[:, 64:P], gwt, start=True, stop=True)
            grep01 = sbh2.tile([2 * 32, NB], f32, tag="grep01")
            nc.scalar.copy(out=grep01, in_=pgrep01)
            grep23 = sbh2.tile([2 * 32, NB], f32, tag="grep23")
            nc.vector.tensor_copy(out=grep23, in_=pgrep23)
            prod01 = sbh2.tile([2 * 32, NB], f32, tag="prod01")
            nc.vector.tensor_mul(out=prod01, in0=peo01, in1=grep01)
            prod23 = sbh2.tile([2 * 32, NB], f32, tag="prod23")
            nc.vector.tensor_mul(out=prod23, in0=peo23, in1=grep23)
            pout = ppsm.tile([P, J, DO], f32, tag="psm")
            for j in range(J):
                nc.tensor.matmul(pout[:, j, :], prod01[:, j * P:(j + 1) * P],
                                 ss0, start=True, stop=False)
                nc.tensor.matmul(pout[:, j, :], prod23[:, j * P:(j + 1) * P],
                                 ss1, start=False, stop=not be3_any)
                if be3_any:
                    nc.tensor.matmul(pout[:, j, :], gwt[:, j * P:(j + 1) * P],
                                     be3s, start=False, stop=True)

            # -------- output layernorm (over DO=8) --------
            stat8 = sbst.tile([P, J, BNS], f32, tag="stat8")
            mv8 = sbst.tile([P, J, BNA], f32, tag="mv8")
            for j in range(J):
                nc.vector.bn_stats(out=stat8[:, j, :], in_=pout[:, j, :])
                nc.vector.bn_aggr(out=mv8[:, j, :], in_=stat8[:, j, :])
            sd8 = sbst.tile([P, J], f32, tag="sd8")
            nc.scalar.activation(out=sd8, in_=mv8[:, :, 1], func=AF.Sqrt,
                                 bias=eps_c, scale=1.0)
            rc8 = sbst.tile([P, J], f32, tag="rc8")
            nc.vector.reciprocal(out=rc8, in_=sd8)
            o_sb = sbsm.tile([P, J, DO], f32, tag="osb")
            for j in range(J):
                nc.vector.tensor_scalar(
                    out=o_sb[:, j, :], in0=pout[:, j, :],
                    scalar1=mv8[:, j, 0:1], scalar2=rc8[:, j:j + 1],
                    op0=ALU.subtract, op1=ALU.mult)
            if out_affine:
                nc.vector.tensor_mul(out=o_sb, in0=o_sb,
                                     in1=gog[:, None, :].broadcast_to((P, J, DO)))
                nc.vector.tensor_add(out=o_sb, in0=o_sb,
                                     in1=gob[:, None, :].broadcast_to((P, J, DO)))
            nc.sync.dma_start(
                out=out_d[b0:b0 + NB, :].rearrange("(j p) o -> p j o", p=P),
                in_=o_sb)

        if repeat > 1:
            with tc.For_i(0, repeat, 1):
                for c in range(n_chunks):
                    chunk_body(c)
        else:
            for c in range(n_chunks):
                chunk_body(c)

    nc.finalize()
    return nc


def _host_arrays(inputs):
    """Build the per-core input map (everything except x, which is sharded)."""
    f = lambda a: np.ascontiguousarray(np.asarray(a, dtype=np.float32))
    Wp = f(inputs["Wp"])
    We3 = f(inputs["We3"])
    # block-diagonal stacked pairs for the third expert layer
    we3bd = np.zeros((E // 2, H1, 32), np.float32)
    for pr in range(E // 2):
        we3bd[pr, 0:H2, 0:DO] = We3[2 * pr]
        we3bd[pr, H2:2 * H2, DO:2 * DO] = We3[2 * pr + 1]
    be2 = f(inputs["be2"])
    be2s = np.zeros((E // 2, P), np.float32)
    for pr in range(E // 2):
        be2s[pr, 0:H2] = be2[2 * pr]
        be2s[pr, H2:2 * H2] = be2[2 * pr + 1]
    rmat = np.zeros((E, P), np.float32)
    smat = np.zeros((P, DO), np.float32)
    for e in range(E):
        base = 64 * (e // 4) + 32 * ((e // 2) % 2) + 8 * (e % 2)
        rmat[e, base:base + DO] = 1.0
        smat[base:base + DO, :] = np.eye(DO, dtype=np.float32)

    ln_in_g = f(inputs["ln_in_g"])
    ln_in_b = f(inputs["ln_in_b"])
    ln_out_g = f(inputs["ln_out_g"])
    ln_out_b = f(inputs["ln_out_b"])
    bg3 = f(inputs["bg3"])
    be3 = f(inputs["be3"])

    in_affine = not (np.all(ln_in_g == 1.0) and np.all(ln_in_b == 0.0))
    bg3_any = bool(np.any(bg3 != 0.0))
    be3_any = bool(np.any(be3 != 0.0))
    out_affine = not (np.all(ln_out_g == 1.0) and np.all(ln_out_b == 0.0))
    flags = (in_affine, bg3_any, be3_any, out_affine)

    m = {
        "wp": Wp,
        "wg1": f(inputs["Wg1"]),
        "wg2": f(inputs["Wg2"]),
        "wg3": f(inputs["Wg3"]),
        "we1": f(inputs["We1"]),
        "we2": f(inputs["We2"]),
        "we3bd": we3bd,
        "bp": f(inputs["bp"]).reshape(D, 1),
        "bg1": f(inputs["bg1"]).reshape(GH, 1),
        "bg2": f(inputs["bg2"]).reshape(GH2, 1),
        "be1": f(inputs["be1"]),
        "be2s": be2s,
        "rmat": rmat,
        "smat": smat,
        "ident": np.eye(P, dtype=np.float32),
    }
    if in_affine:
        m["ging"] = ln_in_g.reshape(1, D)
        m["ginb"] = ln_in_b.reshape(1, D)
    if bg3_any:
        m["bg3"] = bg3.reshape(1, E)
    if be3_any:
        m["be3"] = be3
    if out_affine:
        m["gog"] = ln_out_g.reshape(1, DO)
        m["gob"] = ln_out_b.reshape(1, DO)
    return m, flags


class _Runner:
    """Compiles the program once and exposes a repeatable 8-core launch."""

    def __init__(self, rows, repeat, flags):
        import jax
        import concourse.mybir as mybir
        from concourse.bass2jax import (
            install_neuronx_cc_hook, _bass_exec_p, partition_id_tensor)
        from jax.sharding import Mesh, PartitionSpec, NamedSharding
        from jax.experimental.shard_map import shard_map

        self.jax = jax
        nc = _build_program(rows, repeat, flags)
        install_neuronx_cc_hook()

        in_names, out_names, out_avals, zero_shapes = [], [], [], []
        partition_name = (nc.partition_id_tensor.name
                          if nc.partition_id_tensor else None)
        for alloc in nc.m.functions[0].allocations:
            if not isinstance(alloc, mybir.MemoryLocationSet):
                continue
            name = alloc.memorylocations[0].name
            if alloc.kind == "ExternalInput":
                if name != partition_name:
                    in_names.append(name)
            elif alloc.kind == "ExternalOutput":
                out_names.append(name)
                shape = tuple(alloc.tensor_shape)
                dtype = mybir.dt.np(alloc.dtype)
                out_avals.append(jax.core.ShapedArray(shape, dtype))
                zero_shapes.append((shape, dtype))
        n_params = len(in_names)
        n_outs = len(out_avals)
        in_names_all = list(in_names) + list(out_names) + (
            [partition_name] if partition_name else [])

        def _body(*args):
            operands = list(args)
            if partition_name is not None:
                operands.append(partition_id_tensor())
            outs = _bass_exec_p.bind(
                *operands, out_avals=tuple(out_avals),
                in_names=tuple(in_names_all), out_names=tuple(out_names),
                lowering_input_output_aliases=(),
                sim_require_finite=True, sim_require_nnan=True, nc=nc)
            return tuple(outs)

        devices = jax.devices()[:N_CORES]
        mesh = Mesh(np.asarray(devices), ("core",))
        in_specs = (PartitionSpec("core"),) * (n_params + n_outs)
        out_specs = (PartitionSpec("core"),) * n_outs
        donate = tuple(range(n_params, n_params + n_outs))
        self.fn = jax.jit(
            shard_map(_body, mesh=mesh, in_specs=in_specs,
                      out_specs=out_specs, check_rep=False),
            donate_argnums=donate, keep_unused=True)
        self.sharding = NamedSharding(mesh, PartitionSpec("core"))
        self.in_names = in_names
        self.out_names = out_names
        self.zero_shapes = zero_shapes
        self.rows = rows

    def prepare(self, x_full, common_map):
        """device_put the concatenated inputs once."""
        jax = self.jax
        per_core = []
        for name in self.in_names:
            if name == "x":
                arr = np.ascontiguousarray(
                    np.asarray(x_full, np.float32).reshape(
                        N_CORES, self.rows, D)).reshape(N_CORES * self.rows, D)
            else:
                a = common_map[name]
                arr = np.concatenate([a] * N_CORES, axis=0)
            per_core.append(jax.device_put(arr, self.sharding))
        return per_core

    def run(self, dev_in):
        jax = self.jax
        zeros = [jax.device_put(
            np.zeros((N_CORES * s[0], *s[1:]), d), self.sharding)
            for (s, d) in [(sh, dt) for (sh, dt) in self.zero_shapes]]
        outs = self.fn(*dev_in, *zeros)
        jax.block_until_ready(outs)
        return outs

    def fetch(self, outs):
        res = {}
        for i, name in enumerate(self.out_names):
            a = np.asarray(outs[i])
            res[name] = a
        return res


_RUNNERS = {}


def _get_runner(repeat, flags):
    key = (repeat, flags)
    if key not in _RUNNERS:
        _RUNNERS[key] = _Runner(ROWS, repeat, flags)
    return _RUNNERS[key]


def kernel(**inputs):
    x = np.asarray(inputs["x"], dtype=np.float32)
    assert x.shape == (B, D), x.shape
    common, flags = _host_arrays(inputs)
    runner = _get_runner(1, flags)
    dev_in = runner.prepare(x, common)
    outs = runner.run(dev_in)
    res = runner.fetch(outs)
    out = res["out"].reshape(B, DO)
    gw = res["gw"].reshape(B, DO)
    return out, gw
